# revision 29
# baseline (speedup 1.0000x reference)
"""Trainium2 Bass kernel for the 16-level ternary (Haar-style) wavelet
transform of f (len 3^16) with row-orthonormalized 3x3 Phi matrices.

Strategy:
  - Host: QR-orthonormalize the tiny 3x3 Phi blocks (matches jax CPU QR),
    and precompute combined 4-level 81x81 transform matrices for the tail.
  - Main SPMD kernel (8 cores): f is split into contiguous chunks aligned
    to units of 3^7 = 2187 elements.  Each unit recurses levels 0..6
    entirely inside one SBUF partition (the averaged branch of a unit
    never leaves its partition), so there is zero cross-core and
    cross-partition traffic.  Details DMA straight out per level.
    The 3-term dots are engine-balanced: first multiply on Scalar (ACT),
    accumulates split between Vector (DVE) and GpSimd (Pool).
  - Tail kernel (1 core): the level-7 signal f7 (3^9 = 19683 elems,
    gathered on host between launches) runs levels 7..15 as three
    batched 81x81 matmuls on the Tensor engine (levels 7-10 and 11-14
    each collapse into one precomputed orthogonal 81x81 matrix).
"""

import sys

for _p in ("/opt/trn_rl_repo",):
    if _p not in sys.path:
        sys.path.append(_p)

import numpy as np

import concourse.bass as bass
import concourse.mybir as mybir
import concourse.tile as tile
from concourse.bass_utils import run_bass_kernel_spmd

F32 = mybir.dt.float32
MULT = mybir.AluOpType.mult
ADD = mybir.AluOpType.add

NL = 16                   # total levels
LK = 7                    # levels computed by the main kernel (0..6)
UNIT = 3 ** LK            # 2187 input elems per unit
NUNITS = 3 ** (NL - LK)   # 19683 units overall
NCORES = 8
UPP = 4                   # units per partition per tile
T = 5                     # tiles per core
PAD_UNITS = T * 128 * UPP  # 2560 padded units per core

# contiguous unit ranges per core (2461 x7 + 2456)
_base = [0]
for _k in range(NCORES):
    _base.append(_base[-1] + (2461 if _k < 7 else NUNITS - 7 * 2461))
CORE_U0 = _base[:-1]
CORE_UN = [_base[k + 1] - _base[k] for k in range(NCORES)]

# main-kernel output layout (per core, in elements).  All of a tile's
# level 0-2 details live in one [128, DW] SBUF slab written by compute
# and shipped with ONE 33KB-run DMA per tile; levels 3-6 + f7 coalesce
# into a second staging slab (648B runs).  12 DMAs/rep total.
SLOTD = {}
_s = 0
for _l in range(3):
    SLOTD[_l] = _s
    _s += 2 * UPP * 3 ** (6 - _l)
DW = _s                   # 8424 detail elems per (tile, partition)
OFF_STG = T * 128 * DW
SLOTG = {}
_s = 0
for _l in range(3, LK):
    SLOTG[_l] = _s
    _s += 2 * UPP * 3 ** (6 - _l)
SLOT_F7 = _s
GW = _s + UPP             # 162 staging elems per (tile, partition)
OUT_LEN = OFF_STG + T * 128 * GW

# last tile of each core is only partially populated (units beyond the
# core's range): trim its input/output DMAs to the live partitions
PLIM_LAST = -(-(max(CORE_UN) - (T - 1) * 128 * UPP) // UPP)  # 79

# tail M81 slot layout: [d1_l0 27][d2_l0 27][d1_l0+1 9][d2 9][d1 3][d2 3]
# [d1 1][d2 1][avg 1]
SLOT0 = [0, 54, 72, 78]


def _split_multi_waits(nc):
    """This walrus build rejects any instruction carrying >1 sync wait
    ("Too many sync wait commands").  Split extra waits onto single-wait
    NOPs inserted just before, on the same engine queue (queue order makes
    the semantics identical)."""
    ctr = [0]
    for fn in nc.m.functions:
        for bb in fn.blocks:
            new = []
            for inst in bb.instructions:
                si = inst.sync_info
                if si is not None and si.on_wait and len(si.on_wait) > 1:
                    waits = list(si.on_wait)
                    for w in waits[:-1]:
                        ctr[0] += 1
                        new.append(mybir.InstNoOp(
                            name=f"splitw_{ctr[0]}",
                            engine=inst.engine,
                            bass_nofuse=True,
                            sync_info=mybir.SyncInfo(on_wait=[w], on_update=[]),
                        ))
                    si.on_wait = [waits[-1]]
                new.append(inst)
            bb.instructions = new


def _triple(nc, dst, src, phi_sb, pcol0, engines, tmp=None):
    """dst[p, r] = sum_j src[p, 3r+j] * phi_sb[p, pcol0+j].

    engines = (e0, e1, e2): e0 does the multiply ("act"/"dve"), e1/e2 the
    two fused multiply-accumulates ("dve").  engines == "pool_adds" routes
    the multiplies to ACT and the two adds to GpSimd via tmp tiles."""
    W = src.shape[-1]
    Wo = W // 3
    np_ = src.partition_size()
    x0 = src[:, 0::3]
    x1 = src[:, 1::3]
    x2 = src[:, 2::3]
    c0 = phi_sb[:np_, pcol0 + 0 : pcol0 + 1]
    c1 = phi_sb[:np_, pcol0 + 1 : pcol0 + 2]
    c2 = phi_sb[:np_, pcol0 + 2 : pcol0 + 3]
    assert x0.shape[-1] == Wo and dst.shape[-1] == Wo
    if engines == "pool_adds":
        # ACT materializes all three products; GpSimd does the first add
        # (plain tensor add is all its Q7 firmware supports), DVE the
        # second so the output's last writer is DVE (keeps the ACT-issued
        # store DMA from waiting on the slow Pool engine).
        t1, t2 = tmp
        nc.scalar.mul(dst, x0, c0)
        nc.scalar.mul(t1, x1, c1)
        nc.scalar.mul(t2, x2, c2)
        nc.gpsimd.tensor_add(dst, dst, t1)
        nc.vector.tensor_add(dst, dst, t2)
        return
    e0, e1, e2 = engines
    if e0 == "act":
        nc.scalar.mul(dst, x0, c0)
    else:
        nc.vector.tensor_scalar_mul(dst, x0, c0)
    for e, x, c in ((e1, x1, c1), (e2, x2, c2)):
        assert e == "dve"
        nc.vector.scalar_tensor_tensor(dst, x, c, dst, MULT, ADD)


ENG_DVE = ("act", "dve", "dve")


def build_main(nrep=1, in_bufs=2, copy_only=False, use_pool=False):
    nc = bass.Bass("TRN2", target_bir_lowering=False, debug=False,
                   num_devices=NCORES)
    x = nc.dram_tensor("x", [PAD_UNITS * UNIT], F32, kind="ExternalInput")
    phi = nc.dram_tensor("phi", [128, NL * 9], F32, kind="ExternalInput")
    out = nc.dram_tensor("out", [OUT_LEN], F32, kind="ExternalOutput")

    FW = UPP * UNIT  # 4374 elems per partition per tile

    with tile.TileContext(nc) as tc:
        with (
            tc.tile_pool(name="phi_p", bufs=1) as phi_pool,
            tc.tile_pool(name="in_p", bufs=in_bufs) as in_pool,
            tc.tile_pool(name="a_p", bufs=2) as a_pool,
            tc.tile_pool(name="d_p", bufs=2) as d_pool,
            tc.tile_pool(name="r_p", bufs=1) as r_pool,
            tc.tile_pool(name="m_p", bufs=1) as m_pool,
        ):
            def body():
                phi_sb = phi_pool.tile([128, NL * 9], F32, tag="phi",
                                       name="phi_sb")
                nc.sync.dma_start(phi_sb[:], phi[:])

                # resident buffers for levels 3..6 + staging slab
                R = {3: r_pool.tile([128, T * UPP * 81], F32, tag="R3",
                                    name="R3")}
                for lvl in range(4, 7):
                    R[lvl] = r_pool.tile(
                        [128, T * UPP * 3 ** (6 - lvl) * 3], F32,
                        tag=f"R{lvl}", name=f"R{lvl}")
                STG = r_pool.tile([128, T * GW], F32, tag="STG", name="STG")

                # ---- streamed levels 0..2, one [128, UPP*2187] tile each
                for t in range(T):
                    pl = 128 if t < T - 1 else PLIM_LAST
                    xt = in_pool.tile([128, FW], F32, tag="xt", name="xt")
                    src = bass.AP(x, t * 128 * FW, [[FW, pl], [1, FW]])
                    nc.sync.dma_start(xt[:pl, :], src)

                    if copy_only:
                        dst_off = t * 128 * FW
                        n = min(FW, max(0, (OUT_LEN - dst_off) // 128))
                        if n > 0:
                            nc.scalar.dma_start(
                                bass.AP(out, dst_off, [[n, 128], [1, n]]),
                                xt[:, :n])
                        continue

                    cur = xt[:pl, :]
                    DT = d_pool.tile([128, DW], F32, tag="dt", name="dt")
                    for lvl in range(3):
                        w = 3 ** (6 - lvl)
                        Wo = cur.shape[-1] // 3
                        s0 = SLOTD[lvl]
                        d1_ap = DT[:pl, s0:s0 + Wo]
                        d2_ap = DT[:pl, s0 + Wo:s0 + 2 * Wo]
                        if lvl < 2:
                            av = a_pool.tile([128, Wo], F32, tag=f"a{lvl}",
                                             name=f"a{lvl}")
                            av_ap = av[:pl, :]
                        else:
                            av_ap = R[3][:pl,
                                         t * UPP * 81:(t + 1) * UPP * 81]
                        _triple(nc, av_ap, cur, phi_sb, lvl * 9 + 0, ENG_DVE)
                        _triple(nc, d1_ap, cur, phi_sb, lvl * 9 + 3, ENG_DVE)
                        if lvl == 0 and use_pool:
                            t1 = m_pool.tile([128, Wo], F32, tag="m1",
                                             name="m1")
                            t2 = m_pool.tile([128, Wo], F32, tag="m2",
                                             name="m2")
                            _triple(nc, d2_ap, cur, phi_sb, lvl * 9 + 6,
                                    "pool_adds", tmp=(t1[:pl, :], t2[:pl, :]))
                        else:
                            _triple(nc, d2_ap, cur, phi_sb, lvl * 9 + 6,
                                    ENG_DVE)
                        cur = av_ap
                    # one 33KB-per-partition store for the whole tile
                    nc.scalar.dma_start(
                        bass.AP(out, t * 128 * DW, [[DW, pl], [1, DW]]),
                        DT[:pl, :])

                if copy_only:
                    return

                # ---- batched levels 3..6 straight into the staging slab
                STG3 = STG[:].rearrange("p (t s) -> p t s", t=T)
                for lvl in range(3, LK):
                    w = 3 ** (6 - lvl)
                    uw = UPP * w
                    cur3 = R[lvl][:].rearrange("p (t c) -> p t c", t=T)
                    s0 = SLOTG[lvl]
                    d1_ap = STG3[:, :, s0:s0 + uw]
                    d2_ap = STG3[:, :, s0 + uw:s0 + 2 * uw]
                    if lvl < 6:
                        av_ap = R[lvl + 1][:].rearrange(
                            "p (t c) -> p t c", t=T)
                    else:
                        av_ap = STG3[:, :, SLOT_F7:SLOT_F7 + UPP]
                    x0 = cur3[:, :, 0::3]
                    x1 = cur3[:, :, 1::3]
                    x2 = cur3[:, :, 2::3]
                    for dst, pc in ((av_ap, lvl * 9 + 0), (d1_ap, lvl * 9 + 3),
                                    (d2_ap, lvl * 9 + 6)):
                        c0 = phi_sb[:, pc + 0:pc + 1]
                        c1 = phi_sb[:, pc + 1:pc + 2]
                        c2 = phi_sb[:, pc + 2:pc + 3]
                        nc.scalar.mul(dst, x0, c0)
                        nc.vector.scalar_tensor_tensor(dst, x1, c1, dst,
                                                       MULT, ADD)
                        nc.vector.scalar_tensor_tensor(dst, x2, c2, dst,
                                                       MULT, ADD)

                # one staging DMA: 648B contiguous runs per (t, p) group
                dstg = bass.AP(out, OFF_STG,
                               [[GW, 128], [128 * GW, T], [1, GW]])
                nc.scalar.dma_start(dstg, STG3)

            if nrep == 1:
                body()
            else:
                with tc.For_i(0, nrep, 1):
                    body()

    return nc


def build_tail(nrep=1):
    """Levels 7..15 on the gathered f7 (19683 elems), single core.

    Levels 7-10 and 11-14 each collapse into one 81x81 orthogonal matrix
    (precomputed on host), applied via TensorE matmul after a TensorE
    transpose puts consecutive 81-blocks of f7 on the partition axis.
    """
    nc = bass.Bass("TRN2", target_bir_lowering=False, debug=False,
                   num_devices=1)
    f7 = nc.dram_tensor("f7", [NUNITS], F32, kind="ExternalInput")
    m7 = nc.dram_tensor("m7", [81, 81], F32, kind="ExternalInput")
    m11 = nc.dram_tensor("m11", [81, 81], F32, kind="ExternalInput")
    p15 = nc.dram_tensor("p15", [3, 3], F32, kind="ExternalInput")
    eye = nc.dram_tensor("eye", [81, 81], F32, kind="ExternalInput")
    outt = nc.dram_tensor("tail", [NUNITS], F32, kind="ExternalOutput")

    with tile.TileContext(nc) as tc:
        with (
            tc.tile_pool(name="c_p", bufs=1) as cp,
            tc.tile_pool(name="w_p", bufs=1) as wp,
            tc.psum_pool(name="ps", bufs=1) as pp,
        ):
            def body():
                M7 = cp.tile([81, 81], F32, tag="M7", name="M7")
                M11 = cp.tile([81, 81], F32, tag="M11", name="M11")
                P15 = cp.tile([3, 3], F32, tag="P15", name="P15")
                EYE = cp.tile([81, 81], F32, tag="EYE", name="EYE")
                for sb, dr in ((M7, m7), (M11, m11), (P15, p15), (EYE, eye)):
                    nc.sync.dma_start(sb[:], dr[:])

                X7 = wp.tile([81, 243], F32, tag="X7", name="X7")
                nc.sync.dma_start(X7[:],
                                  bass.AP(f7, 0, [[243, 81], [1, 243]]))

                F11 = wp.tile([81, 3], F32, tag="F11", name="F11")
                # blocks c = 3p + b; levels 7-10 via M7 on each 81-block
                for b in range(3):
                    Pb = pp.tile([81, 81], F32, tag="Pb", name=f"Pb{b}")
                    nc.tensor.transpose(Pb[:], X7[:, 81 * b:81 * (b + 1)],
                                        EYE[:])
                    Sb = wp.tile([81, 81], F32, tag="Sb", name=f"Sb{b}")
                    nc.scalar.copy(Sb[:], Pb[:])
                    Qb = pp.tile([81, 81], F32, tag="Qb", name=f"Qb{b}")
                    nc.tensor.matmul(Qb[:], M7[:], Sb[:], start=True,
                                     stop=True)
                    Rb = wp.tile([81, 81], F32, tag="Rb", name=f"Rb{b}")
                    nc.scalar.copy(Rb[:], Qb[:])
                    Tb = pp.tile([81, 81], F32, tag="Tb", name=f"Tb{b}")
                    nc.tensor.transpose(Tb[:], Rb[:], EYE[:])
                    Ub = wp.tile([81, 81], F32, tag="Ub", name=f"Ub{b}")
                    nc.scalar.copy(Ub[:], Tb[:])
                    # details: Ub[p, slot] -> f_hat[r*3^(15-l) + w*(3p+b) + m]
                    for li, lvl in enumerate(range(7, 11)):
                        w = 3 ** (3 - li)
                        base = 3 ** (15 - lvl)
                        c0 = SLOT0[li]
                        for r in range(2):
                            dst = bass.AP(outt, (r + 1) * base + w * b,
                                          [[3 * w, 81], [1, w]])
                            nc.sync.dma_start(
                                dst, Ub[:, c0 + r * w:c0 + (r + 1) * w])
                    nc.scalar.copy(F11[:, b:b + 1], Ub[:, 80:81])

                # f11[3p+b] = F11[p, b] -> contiguous [1, 243]
                F11L = wp.tile([1, 243], F32, tag="F11L", name="F11L")
                nc.sync.dma_start(F11L[:], F11[:])

                # levels 11-14 via M11 on the three 81-blocks of f11
                S2 = wp.tile([81, 3], F32, tag="S2", name="S2")
                for b2 in range(3):
                    P2 = pp.tile([81, 1], F32, tag="P2", name=f"P2{b2}")
                    nc.tensor.transpose(
                        P2[:], F11L[:, 81 * b2:81 * (b2 + 1)], EYE[0:1, 0:1])
                    nc.scalar.copy(S2[:, b2:b2 + 1], P2[:])
                Q2 = pp.tile([81, 3], F32, tag="Q2", name="Q2")
                nc.tensor.matmul(Q2[:], M11[:], S2[:], start=True, stop=True)
                R2 = wp.tile([81, 3], F32, tag="R2", name="R2")
                nc.scalar.copy(R2[:], Q2[:])
                T2 = pp.tile([3, 81], F32, tag="T2", name="T2")
                nc.tensor.transpose(T2[:], R2[:], EYE[:])
                U2 = wp.tile([3, 81], F32, tag="U2", name="U2")
                nc.scalar.copy(U2[:], T2[:])
                for li, lvl in enumerate(range(11, 15)):
                    w = 3 ** (3 - li)
                    base = 3 ** (15 - lvl)
                    c0 = SLOT0[li]
                    for r in range(2):
                        dst = bass.AP(outt, (r + 1) * base,
                                      [[w, 3], [1, w]])
                        nc.sync.dma_start(
                            dst, U2[:, c0 + r * w:c0 + (r + 1) * w])

                # level 15: f15 = U2[:, 80]; [fhat0, d1_15, d2_15] = f15.T@P15
                F15 = wp.tile([3, 1], F32, tag="F15", name="F15")
                nc.scalar.copy(F15[:], U2[:, 80:81])
                Q3 = pp.tile([1, 3], F32, tag="Q3", name="Q3")
                nc.tensor.matmul(Q3[:], F15[:], P15[:], start=True, stop=True)
                U3 = wp.tile([1, 3], F32, tag="U3", name="U3")
                nc.scalar.copy(U3[:], Q3[:])
                nc.sync.dma_start(bass.AP(outt, 0, [[3, 1], [1, 3]]), U3[:])

            if nrep == 1:
                body()
            else:
                with tc.For_i(0, nrep, 1):
                    body()

    return nc


def _phi_from_inputs(Phi_P: np.ndarray) -> np.ndarray:
    Q = np.stack([np.linalg.qr(Phi_P[i].T.astype(np.float32))[0]
                  for i in range(Phi_P.shape[0])])
    return np.transpose(Q, (0, 2, 1)).astype(np.float32)


def _m81(phis: np.ndarray) -> np.ndarray:
    """Combined 4-level transform matrix (81x81), float64 accumulation.

    Output slots per 81-block: [d1_l0 (27)][d2_l0 (27)][d1_l0+1 (9)]
    [d2 (9)][d1 (3)][d2 (3)][d1 (1)][d2 (1)][avg (1)]."""
    phis = phis.astype(np.float64)
    M = np.zeros((81, 81), np.float64)
    cur = np.eye(81, dtype=np.float64)          # [cur_len, 81]
    for li in range(4):
        fm = cur.reshape(-1, 3, 81)
        d1 = np.einsum("j,kjb->kb", phis[li][1], fm)
        d2 = np.einsum("j,kjb->kb", phis[li][2], fm)
        av = np.einsum("j,kjb->kb", phis[li][0], fm)
        w = d1.shape[0]
        M[SLOT0[li]:SLOT0[li] + w] = d1
        M[SLOT0[li] + w:SLOT0[li] + 2 * w] = d2
        cur = av
    M[80] = cur[0]
    return M.astype(np.float32)


def _tail_inputs(Phi: np.ndarray) -> dict:
    m7 = _m81(Phi[7:11]).T.copy()       # lhsT: matmul computes lhsT.T @ rhs
    m11 = _m81(Phi[11:15]).T.copy()
    p15 = Phi[15].T.copy()              # p15[j, r] = Phi[15, r, j]
    return {"m7": np.ascontiguousarray(m7),
            "m11": np.ascontiguousarray(m11),
            "p15": np.ascontiguousarray(p15),
            "eye": np.eye(81, dtype=np.float32)}


_CACHE = {}


def kernel(f: np.ndarray, Phi_P: np.ndarray) -> np.ndarray:
    f = np.asarray(f, dtype=np.float32).ravel()
    Phi = _phi_from_inputs(np.asarray(Phi_P, dtype=np.float32))

    phi_all = np.broadcast_to(
        Phi.reshape(1, NL * 9), (128, NL * 9)).copy()

    if "main" not in _CACHE:
        _CACHE["main"] = build_main()
        _split_multi_waits(_CACHE["main"])
        _CACHE["tail"] = build_tail()
        _split_multi_waits(_CACHE["tail"])

    in_maps = []
    for k in range(NCORES):
        lo = CORE_U0[k] * UNIT
        n = CORE_UN[k] * UNIT
        xk = np.zeros(PAD_UNITS * UNIT, dtype=np.float32)
        xk[:n] = f[lo:lo + n]
        in_maps.append({"x": xk, "phi": phi_all})

    res = run_bass_kernel_spmd(_CACHE["main"], in_maps, list(range(NCORES)))

    f_hat = np.empty(3 ** NL, dtype=np.float32)
    f7g = np.empty(NUNITS, dtype=np.float32)
    for k in range(NCORES):
        ok = res.results[k]["out"]
        u0, un = CORE_U0[k], CORE_UN[k]
        td = ok[:T * 128 * DW].reshape(T * 128, DW)
        for i in range(3):
            w = 3 ** (6 - i)
            uw = UPP * w
            base = 3 ** (15 - i)
            s0 = SLOTD[i]
            d1v = td[:, s0:s0 + uw].reshape(-1, w)
            d2v = td[:, s0 + uw:s0 + 2 * uw].reshape(-1, w)
            f_hat[base + u0 * w: base + (u0 + un) * w] = d1v[:un].ravel()
            f_hat[2 * base + u0 * w: 2 * base + (u0 + un) * w] = \
                d2v[:un].ravel()
        stg = ok[OFF_STG: OFF_STG + T * 128 * GW].reshape(T * 128, GW)
        for lvl in range(3, LK):
            w = 3 ** (6 - lvl)
            uw = UPP * w
            base = 3 ** (15 - lvl)
            s0 = SLOTG[lvl]
            d1v = stg[:, s0:s0 + uw].reshape(-1, w)
            d2v = stg[:, s0 + uw:s0 + 2 * uw].reshape(-1, w)
            f_hat[base + u0 * w: base + (u0 + un) * w] = d1v[:un].ravel()
            f_hat[2 * base + u0 * w: 2 * base + (u0 + un) * w] = \
                d2v[:un].ravel()
        f7g[u0:u0 + un] = stg[:, SLOT_F7:].reshape(-1)[:un]

    tin = {"f7": f7g, **_tail_inputs(Phi)}
    rest = run_bass_kernel_spmd(_CACHE["tail"], [tin], [0])
    f_hat[:NUNITS] = rest.results[0]["tail"]
    return f_hat
